# revision 63
# baseline (speedup 1.0000x reference)
"""Trainium2 Bass kernel for MultiHeadAttention + residual + BatchNorm.

Model (reference):
  q = query @ Wq.T ; k = key @ Wk.T ; v = key @ Wv.T    (per-head split)
  score = q k^T / sqrt(D), causal mask, softmax over keys
  res   = (attn @ v) + query
  out   = batchnorm(res over all (N*L) rows, per feature) * gamma + beta

Sharding over 8 cores: core c -> (batch n = c % 4, head-block hb = c // 4).
Each core computes its batch's 8 heads (512 of the 1024 features).
BatchNorm statistics are all-reduced across the 4 cores sharing a head
block (replica groups [[0,1,2,3],[4,5,6,7]]).

The host passes query/key and the W slices pre-transposed (a sharding
layout choice): qT/kT are [D, L], W^T slices are [D, F].

Pipeline: one fused loop over 512-row chunks. For chunk ic:
  A(ic): project q/k/v for rows [512ic, 512ic+512)   (PE f32r matmuls)
  B(ic): causal attention for queries in the chunk — scores transposed
         [j, i] via matmul, exp on ScalarE, [V|1] matmul accumulates
         OT[p, i] + rowsums
  C(ic): PE-transpose OT back to [l, d]; fused normalize+residual
         (scalar_tensor_tensor); per-feature sums via ones-matmuls;
         res rows stream to DRAM
Then one AllReduce of the BN stats, and a BN-apply pass re-reading res
(the re-read DMAs overlap the collective).
"""

import math
import sys

sys.path.insert(0, "/opt/trn_rl_repo")

import numpy as np

import concourse.bass as bass
import concourse.mybir as mybir
from concourse import bacc
import concourse.tile as tile
from concourse import bass_utils
from concourse.masks import make_identity

F32 = mybir.dt.float32
F32R = mybir.dt.float32r

N = 4
L = 2048
D = 1024
H = 16
P = 64
NCORES = 8
NB = 4            # batches
HBS = 2           # head blocks
F = D // HBS      # features per core = 512
H8 = H // HBS     # heads per core = 8
EPS = 1e-5
SCALE = 1.0 / math.sqrt(D)

_cached = {}


def r(ap):
    return ap.bitcast(F32R)


def build_program(l=L):
    """Build the SPMD Bass program (identical on all 8 cores)."""
    lc_n = l // 512        # 512-wide l chunks
    ls_n = l // 128        # 128-wide l chunks
    ic_n = l // 512
    nrows_total = float(NB * l)   # rows in the global batch-norm

    nc = bacc.Bacc("TRN2", target_bir_lowering=False, debug=False,
                   num_devices=NCORES)

    qt_nd = nc.dram_tensor("qt_nd", [D, l], F32, kind="ExternalInput").ap()
    kt_nd = nc.dram_tensor("kt_nd", [D, l], F32, kind="ExternalInput").ap()
    wqt = nc.dram_tensor("wqt", [D, F], F32, kind="ExternalInput").ap()
    wkt = nc.dram_tensor("wkt", [D, F], F32, kind="ExternalInput").ap()
    wvt = nc.dram_tensor("wvt", [D, F], F32, kind="ExternalInput").ap()
    q_res = nc.dram_tensor("q_res", [l, F], F32, kind="ExternalInput").ap()
    gamma = nc.dram_tensor("gamma", [1, F], F32, kind="ExternalInput").ap()
    beta = nc.dram_tensor("beta", [1, F], F32, kind="ExternalInput").ap()
    out_s = nc.dram_tensor("out_s", [l, F], F32, kind="ExternalOutput").ap()

    with tile.TileContext(nc) as tc, \
         tc.tile_pool(name="consts", bufs=1) as consts, \
         tc.tile_pool(name="persist", bufs=1) as persist, \
         tc.tile_pool(name="wt", bufs=1) as wtp, \
         tc.tile_pool(name="qtp", bufs=2) as qtp, \
         tc.tile_pool(name="xt", bufs=8) as xtp, \
         tc.tile_pool(name="attnp", bufs=1) as attnp, \
         tc.tile_pool(name="at", bufs=4) as atp, \
         tc.tile_pool(name="qin", bufs=3) as qinp, \
         tc.tile_pool(name="sq", bufs=2) as sqp, \
         tc.tile_pool(name="resp", bufs=2) as resp, \
         tc.tile_pool(name="outp", bufs=3) as outp, \
         tc.tile_pool(name="bnp", bufs=1) as bnp, \
         tc.tile_pool(name="small", bufs=8) as smallp, \
         tc.tile_pool(name="spsum", bufs=1, space="PSUM") as spsum, \
         tc.tile_pool(name="pja", bufs=1, space="PSUM") as pja, \
         tc.tile_pool(name="stp", bufs=3, space="PSUM") as stpp, \
         tc.tile_pool(name="otp", bufs=2, space="PSUM") as otpp, \
         tc.tile_pool(name="dram", bufs=1, space="DRAM") as dramp:

        identity = consts.tile([128, 128], F32)
        make_identity(nc, identity)
        ones_col = consts.tile([128, 1], F32)
        nc.vector.memset(ones_col, 1.0)
        eps_sb = consts.tile([128, 1], F32)
        nc.vector.memset(eps_sb, EPS)
        # gamma/beta in [128 p, 4 oc] layout (feature f = oc*128 + p)
        gamma_sb = consts.tile([128, 4], F32)
        nc.sync.dma_start(gamma_sb, bass.AP(
            tensor=gamma.tensor, offset=gamma.offset, ap=[[1, 128], [128, 4]]))
        beta_sb = consts.tile([128, 4], F32)
        nc.sync.dma_start(beta_sb, bass.AP(
            tensor=beta.tensor, offset=beta.offset, ap=[[1, 128], [128, 4]]))

        kt_sb = persist.tile([128, 4 * l], F32, tag="kt")
        v_sb = persist.tile([128, ls_n * 520], F32, tag="v")
        nc.gpsimd.memset(v_sb, 1.0)  # bakes the ones columns
        res_dram = dramp.tile([l, F], F32, tag="res_dram")

        # W^T[d, o] tiles, direct DMA (inputs are pre-transposed).
        # dc-interleaved emission so the first projection group's weight
        # chunks arrive before the later chunks of other matrices.
        wts = {}
        wdrams = {"wqt": wqt, "wkt": wkt, "wvt": wvt}
        for wname in ("wqt", "wkt", "wvt"):
            wts[wname] = wtp.tile([128, 8 * F], F32, tag=wname, name=wname)
        for dc in range(8):
            for wname in ("wqt", "wkt", "wvt"):
                nc.sync.dma_start(
                    r(wts[wname][:, dc * F:(dc + 1) * F]),
                    r(wdrams[wname][dc * 128:(dc + 1) * 128, :]))

        sum_ps = spsum.tile([1, 512], F32, tag="sum")
        sq_ps = spsum.tile([1, 512], F32, tag="sq")

        def emit_A(ic):
            lc = ic
            # ---------- A(ic): projections for rows [512ic, 512ic+512) --
            if True:
                qt_ic = qtp.tile([128, 4 * 512], F32, tag="qt", name="qt_ic")
                for side in ("q", "k"):
                    src = qt_nd if side == "q" else kt_nd
                    xts = []
                    for dc in range(8):
                        xt_t = xtp.tile([128, 512], F32, tag="xt",
                                        name="xt_t")
                        nc.sync.dma_start(
                            r(xt_t),
                            r(src[dc * 128:(dc + 1) * 128,
                                  lc * 512:(lc + 1) * 512]))
                        xts.append(xt_t)

                    if side == "q":
                        wt_use = wts["wqt"]
                    else:
                        wt_use = wts["wkt"]
                    for oc in range(4):
                        pj = pja.tile([128, 512], F32, tag="pj", name="pj")
                        for dc in range(8):
                            nc.tensor.matmul(
                                pj,
                                r(wt_use[:, dc * F + oc * 128:
                                         dc * F + oc * 128 + 128]),
                                r(xts[dc]),
                                start=(dc == 0), stop=(dc == 7))
                        if side == "q":
                            nc.vector.tensor_copy(
                                r(qt_ic[:, oc * 512:(oc + 1) * 512]), pj)
                        else:
                            nc.vector.tensor_copy(
                                r(kt_sb[:, oc * l + lc * 512:
                                        oc * l + lc * 512 + 512]), pj)
                    if side == "k":
                        # V[l, o] per 128-row chunk (key's XT as lhsT)
                        for lsub in range(4):
                            pj = pja.tile([128, 512], F32, tag="pj",
                                          name="pj")
                            for dc in range(8):
                                nc.tensor.matmul(
                                    pj,
                                    r(xts[dc][:, lsub * 128:
                                              lsub * 128 + 128]),
                                    r(wts["wvt"][:, dc * F:dc * F + 512]),
                                    start=(dc == 0), stop=(dc == 7))
                            jc = lc * 4 + lsub
                            vdst = v_sb[:, jc * 520:(jc + 1) * 520]
                            vdst = vdst.rearrange(
                                "p (h x) -> p h x", h=8)[:, :, 0:64]
                            vsrc = pj.rearrange("p (h x) -> p h x", h=8)
                            nc.vector.tensor_copy(r(vdst), vsrc)
            return qt_ic

        def emit_B(ic, qt_ic):
            # ---------- B(ic): attention for this query chunk -----------
            attn_ic = attnp.tile([65, H8 * 512], F32, tag="attn",
                                 name="attn_ic")
            jmax = 4 * ic + 4
            if True:
                for h8 in range(H8):
                    po = (h8 % 2) * 64
                    co = (h8 // 2) * l
                    ot = otpp.tile([65, 512], F32, tag="ot", name="ot")
                    for jc in range(jmax):
                        st = stpp.tile([128, 512], F32, tag="st", name="st")
                        nc.tensor.matmul(
                            st,
                            r(kt_sb[po:po + 64,
                                    co + jc * 128:co + jc * 128 + 128]),
                            r(qt_ic[po:po + 64,
                                    (h8 // 2) * 512:(h8 // 2) * 512 + 512]),
                            start=True, stop=True)
                        at = atp.tile([128, 512], F32, tag="at", name="at")
                        rr = jc - 4 * ic
                        if rr < 0:
                            nc.scalar.activation(
                                r(at), st,
                                mybir.ActivationFunctionType.Exp,
                                scale=SCALE)
                        else:
                            if rr > 0:
                                nc.gpsimd.memset(at[:, 0:rr * 128], 0.0)
                            nc.scalar.activation(
                                r(at[:, rr * 128:512]),
                                st[:, rr * 128:512],
                                mybir.ActivationFunctionType.Exp,
                                scale=SCALE)
                            # keep j <= i inside the diagonal block
                            nc.gpsimd.affine_select(
                                out=r(at[:, rr * 128:(rr + 1) * 128]),
                                in_=r(at[:, rr * 128:(rr + 1) * 128]),
                                compare_op=mybir.AluOpType.is_ge,
                                fill=0.0,
                                base=0,
                                pattern=[[1, 128]],
                                channel_multiplier=-1,
                            )
                        nc.tensor.matmul(
                            ot,
                            r(v_sb[:, jc * 520 + h8 * 65:
                                   jc * 520 + h8 * 65 + 65]),
                            r(at),
                            start=(jc == 0), stop=(jc == jmax - 1))
                    nc.vector.tensor_copy(
                        attn_ic[:, h8 * 512:(h8 + 1) * 512], ot)
            return attn_ic

        def emit_C(ic, attn_ic):
            # ---------- C(ic): transpose + residual + stats -------------
            if True:
                for t in range(4):
                    ls = ic * 4 + t
                    qtile = qinp.tile([128, F], F32, tag="q", name="qtile")
                    nc.sync.dma_start(qtile,
                                      q_res[ls * 128:(ls + 1) * 128, :])
                    rtile = resp.tile([128, F], F32, tag="res", name="rtile")
                    for h8 in range(H8):
                        tp = stpp.tile([128, 65], F32, tag="st", name="tp")
                        nc.tensor.transpose(
                            tp,
                            attn_ic[:, h8 * 512 + t * 128:
                                    h8 * 512 + t * 128 + 128],
                            identity[0:65, 0:65])
                        rec = smallp.tile([128, 1], F32, tag="rec",
                                          name="rec")
                        nc.vector.reciprocal(rec, tp[:, 64:65])
                        # res = attn/rowsum + query   (fused in one op)
                        nc.vector.scalar_tensor_tensor(
                            out=r(rtile[:, h8 * 64:(h8 + 1) * 64]),
                            in0=tp[:, 0:64],
                            scalar=rec,
                            in1=qtile[:, h8 * 64:(h8 + 1) * 64],
                            op0=mybir.AluOpType.mult,
                            op1=mybir.AluOpType.add)
                    # per-feature sums over rows via ones-matmuls
                    sqt = sqp.tile([128, F], F32, tag="sq", name="sqt")
                    nc.scalar.activation(
                        r(sqt), rtile, mybir.ActivationFunctionType.Square)
                    nc.tensor.matmul(
                        sum_ps, r(ones_col), r(rtile),
                        start=(ls == 0), stop=(ls == ls_n - 1),
                        skip_group_check=True)
                    nc.tensor.matmul(
                        sq_ps, r(ones_col), r(sqt),
                        start=(ls == 0), stop=(ls == ls_n - 1),
                        skip_group_check=True)
                    nc.sync.dma_start(res_dram[ls * 128:(ls + 1) * 128, :],
                                      rtile)

        # software pipeline: projections run one chunk ahead of attention
        qt_next = emit_A(0)
        for ic in range(ic_n):
            qt_cur = qt_next
            if ic + 1 < ic_n:
                qt_next = emit_A(ic + 1)
            attn_ic = emit_B(ic, qt_cur)
            emit_C(ic, attn_ic)

        # ---------------- collective + BN ------------------------------
        cc_in = dramp.tile([1, 2 * F], F32, tag="cc_in")
        cc_out = dramp.tile([4, 2 * F], F32, tag="cc_out")

        def dview(dtile, off):
            return bass.AP(tensor=dtile.tensor,
                           offset=dtile.offset + off,
                           ap=[[1, 128], [128, 4]])

        sums_sb = bnp.tile([1, 512], F32, tag="sums", name="sums")
        nc.vector.tensor_copy(sums_sb, sum_ps)
        sqs_sb = bnp.tile([1, 512], F32, tag="sqs", name="sqs")
        nc.vector.tensor_copy(sqs_sb, sq_ps)
        nc.sync.dma_start(cc_in[:, 0:F], sums_sb)
        nc.sync.dma_start(cc_in[:, F:2 * F], sqs_sb)

        nc.gpsimd.collective_compute(
            "AllGather",
            mybir.AluOpType.bypass,
            replica_groups=[[0, 1, 2, 3], [4, 5, 6, 7]],
            ins=[cc_in],
            outs=[cc_out],
        )

        def gview(off):
            return bass.AP(tensor=cc_out.tensor,
                           offset=cc_out.offset + off,
                           ap=[[1, 128], [128, 4]])

        gsum4 = bnp.tile([128, 4, 4], F32, tag="gsum4", name="gsum4")
        gsq4 = bnp.tile([128, 4, 4], F32, tag="gsq4", name="gsq4")
        for rank in range(4):
            nc.sync.dma_start(gsum4[:, :, rank], gview(rank * 2 * F))
            nc.sync.dma_start(gsq4[:, :, rank], gview(rank * 2 * F + F))
        gsum = bnp.tile([128, 4], F32, tag="gsum", name="gsum")
        nc.vector.reduce_sum(gsum, gsum4, axis=mybir.AxisListType.X)
        gsq = bnp.tile([128, 4], F32, tag="gsq", name="gsq")
        nc.vector.reduce_sum(gsq, gsq4, axis=mybir.AxisListType.X)

        mean = bnp.tile([128, 4], F32, tag="mean", name="mean")
        nc.vector.tensor_scalar_mul(mean, gsum, 1.0 / nrows_total)
        ex2 = bnp.tile([128, 4], F32, tag="ex2", name="ex2")
        nc.vector.tensor_scalar_mul(ex2, gsq, 1.0 / nrows_total)
        m2 = bnp.tile([128, 4], F32, tag="m2", name="m2")
        nc.vector.tensor_mul(m2, mean, mean)
        var = bnp.tile([128, 4], F32, tag="var", name="var")
        nc.vector.tensor_sub(var, ex2, m2)
        std = bnp.tile([128, 4], F32, tag="std", name="std")
        nc.scalar.activation(std, var,
                             mybir.ActivationFunctionType.Sqrt,
                             bias=eps_sb)
        rstd = bnp.tile([128, 4], F32, tag="rstd", name="rstd")
        nc.vector.reciprocal(rstd, std)
        gp = bnp.tile([128, 4], F32, tag="gp", name="gp")
        nc.vector.tensor_mul(gp, gamma_sb, rstd)
        mgp = bnp.tile([128, 4], F32, tag="mgp", name="mgp")
        nc.vector.tensor_mul(mgp, mean, gp)
        bp = bnp.tile([128, 4], F32, tag="bp", name="bp")
        nc.vector.tensor_sub(bp, beta_sb, mgp)

        # broadcast gp/bp over partitions: bounce via DRAM, then a
        # partition-step-0 DMA read
        gp_dram = dramp.tile([1, F], F32, tag="gp_dram")
        bp_dram = dramp.tile([1, F], F32, tag="bp_dram")
        nc.sync.dma_start(dview(gp_dram, 0), gp)
        nc.sync.dma_start(dview(bp_dram, 0), bp)
        gbc = bnp.tile([128, F], F32, tag="gbcs", name="gbcs")
        nc.sync.dma_start(gbc, bass.AP(
            tensor=gp_dram.tensor, offset=gp_dram.offset,
            ap=[[0, 128], [1, F]]))
        bbc = bnp.tile([128, F], F32, tag="bbcs", name="bbcs")
        nc.sync.dma_start(bbc, bass.AP(
            tensor=bp_dram.tensor, offset=bp_dram.offset,
            ap=[[0, 128], [1, F]]))

        for ls in range(ls_n):
            rt2 = outp.tile([128, F], F32, tag="rt2", name="rt2", bufs=4)
            nc.sync.dma_start(rt2, res_dram[ls * 128:(ls + 1) * 128, :])
            t1 = outp.tile([128, F], F32, tag="t1", name="t1", bufs=2)
            t2 = outp.tile([128, F], F32, tag="t2", name="t2", bufs=2)
            # independent halves on DVE and GpSimd (parallel pipelines)
            nc.vector.tensor_mul(t1[:, 0:256], rt2[:, 0:256], gbc[:, 0:256])
            nc.vector.tensor_add(t2[:, 0:256], t1[:, 0:256], bbc[:, 0:256])
            nc.gpsimd.tensor_mul(t1[:, 256:512], rt2[:, 256:512],
                                 gbc[:, 256:512])
            nc.gpsimd.tensor_add(t2[:, 256:512], t1[:, 256:512],
                                 bbc[:, 256:512])
            nc.sync.dma_start(out_s[ls * 128:(ls + 1) * 128, :], t2)

    nc.compile()
    return nc


def get_runner(nc):
    """Build (once) a cached jitted SPMD executor for the Bass program."""
    if "runner" in _cached:
        return _cached["runner"]

    import jax
    from jax.experimental.shard_map import shard_map
    from jax.sharding import Mesh, PartitionSpec
    from concourse import bass2jax

    bass2jax.install_neuronx_cc_hook()

    partition_name = (nc.partition_id_tensor.name
                      if nc.partition_id_tensor else None)
    in_names, out_names, out_avals, zero_outs = [], [], [], []
    for alloc in nc.m.functions[0].allocations:
        if not isinstance(alloc, mybir.MemoryLocationSet):
            continue
        name = alloc.memorylocations[0].name
        if alloc.kind == "ExternalInput":
            if name != partition_name:
                in_names.append(name)
        elif alloc.kind == "ExternalOutput":
            shape = tuple(alloc.tensor_shape)
            dtype = mybir.dt.np(alloc.dtype)
            out_names.append(name)
            out_avals.append(jax.core.ShapedArray(shape, dtype))
            zero_outs.append(np.zeros(shape, dtype))
    n_params = len(in_names)
    n_outs = len(out_avals)
    all_names = in_names + out_names
    if partition_name is not None:
        all_names = all_names + [partition_name]

    def _body(*args):
        operands = list(args)
        if partition_name is not None:
            operands.append(bass2jax.partition_id_tensor())
        outs = bass2jax._bass_exec_p.bind(
            *operands,
            out_avals=tuple(out_avals),
            in_names=tuple(all_names),
            out_names=tuple(out_names),
            lowering_input_output_aliases=(),
            sim_require_finite=True,
            sim_require_nnan=True,
            nc=nc,
        )
        return tuple(outs)

    devices = jax.devices()[:NCORES]
    mesh = Mesh(np.asarray(devices), ("core",))
    in_specs = (PartitionSpec("core"),) * (n_params + n_outs)
    out_specs = (PartitionSpec("core"),) * n_outs
    donate = tuple(range(n_params, n_params + n_outs))
    sharded = jax.jit(
        shard_map(_body, mesh=mesh, in_specs=in_specs, out_specs=out_specs,
                  check_rep=False),
        donate_argnums=donate, keep_unused=True)

    def run_np(in_maps):
        concat_in = [
            np.concatenate([np.asarray(in_maps[c][nm]) for c in range(NCORES)],
                           axis=0)
            for nm in in_names]
        concat_zeros = [np.zeros((NCORES * z.shape[0], *z.shape[1:]), z.dtype)
                        for z in zero_outs]
        out_arrs = sharded(*concat_in, *concat_zeros)
        return [
            {nm: np.asarray(out_arrs[i]).reshape(
                NCORES, *out_avals[i].shape)[c]
             for i, nm in enumerate(out_names)}
            for c in range(NCORES)]

    _cached["runner"] = (run_np, sharded, in_names, out_names, out_avals,
                         zero_outs, mesh)
    return _cached["runner"]


def make_in_maps(inputs, l):
    query = np.asarray(inputs["query"], dtype=np.float32)
    key = np.asarray(inputs["key"], dtype=np.float32)
    Wq = np.asarray(inputs["Wq"], dtype=np.float32)
    Wk = np.asarray(inputs["Wk"], dtype=np.float32)
    Wv = np.asarray(inputs["Wv"], dtype=np.float32)
    gamma = np.asarray(inputs["gamma"], dtype=np.float32)
    beta = np.asarray(inputs["beta"], dtype=np.float32)

    in_maps = []
    for c in range(NCORES):
        n, hb = c % NB, c // NB
        sl = slice(hb * F, (hb + 1) * F)
        in_maps.append({
            "qt_nd": np.ascontiguousarray(query[n].T),
            "kt_nd": np.ascontiguousarray(key[n].T),
            "wqt": np.ascontiguousarray(Wq[sl].T),
            "wkt": np.ascontiguousarray(Wk[sl].T),
            "wvt": np.ascontiguousarray(Wv[sl].T),
            "q_res": np.ascontiguousarray(query[n][:, sl]),
            "gamma": np.ascontiguousarray(gamma[sl].reshape(1, F)),
            "beta": np.ascontiguousarray(beta[sl].reshape(1, F)),
        })
    return in_maps


def kernel(**inputs):
    l = np.asarray(inputs["query"]).shape[1]
    if "nc" not in _cached or _cached.get("l") != l:
        _cached["nc"] = build_program(l)
        _cached["l"] = l
    nc = _cached["nc"]

    in_maps = make_in_maps(inputs, l)
    run_np = get_runner(nc)[0]
    results = run_np(in_maps)

    out = np.zeros((N, l, D), dtype=np.float32)
    for c in range(NCORES):
        n, hb = c % NB, c // NB
        out[n, :, hb * F:(hb + 1) * F] = results[c]["out_s"]
    return out
